# revision 47
# baseline (speedup 1.0000x reference)
"""Trainium2 Bass kernel for AnchorGNNPocket (GNN message passing), sparse.

Data-parallel over batch B=8: one complex per NeuronCore. The cutoff graph is
only ~12% dense (7.3k-8.7k real edges of 65536 pairs), so instead of the dense
[N,N,H] edge sweep, each sample's edges are sorted by destination row i and
padded per-row to a multiple of 32 ("slots"); all samples pad to a fixed
S_MAX=384 slots (EP=12288 edge columns) so every core runs the identical
instruction stream and only tensor *contents* differ per sample.

Per [128, 512] edge tile (16 slots), the stages are
  pre  = gather(ha,row) + gather(hb,col) + wc (x) d2 + be1
         -- ONE PSUM group: K=19 matmul (slot-packed ha rows 0-15 + wc hi/lo
         rows 16-18, against a static ones/d2 pattern) + 2 K=128 one-hot
         col gathers; all matmuls stay in PE quadrant q0 (mixed-quadrant
         lhsT tiles measurably break the PE load/stream overlap)
  m    = relu(We2^T relu(pre+be1) + be2)
  att  = WatFull^T m + adj_logit (K=1) + bat
  sigp = sigmoid(att)        (pad cols carry -1e9 logits -> sigp == 0)
  aggS[:, s] = sum_e (m/NORM)*sigp per slot   (DVE stst accum_out)
and the five stages are emitted SKEWED (pre(t), relu1(t-1), m(t-2), att(t-3),
gating(t-4)) so every engine always has independent work in flight.

Slot sums combine to rows via PE transposes of aggS + a one-hot R[s->i]
matmul at layer end; layer-0 node projections (ha/hb/slot-gather) are
precomputed on host to shorten the startup critical path.
"""

import os
import sys

import numpy as np

if not any(os.path.isdir(os.path.join(p, "concourse")) for p in sys.path if p):
    sys.path.insert(0, "/opt/trn_rl_repo")

# ---- problem constants (hardcoded per contest rules) ----
B, NS, NP = 8, 32, 224
N = NS + NP                      # 256 nodes
LIG_NF, POK_NF, JNF, HID, OUT_NF, NLAYERS = 10, 25, 32, 128, 128, 4
CUT2 = 4.5 ** 2
NORM = 100.0

_F32 = np.float32
SLOT = 32                        # edge columns per slot (one row per slot)
S_MAX = 384                      # padded slot count (shared by all cores)
EP = S_MAX * SLOT                # 12288 edge columns
TILE = 512                       # edge columns per PSUM tile
SPT = TILE // SLOT               # 16 slots per tile
NT = EP // TILE                  # 24 tiles


def _np_silu(x):
    return x / (1.0 + np.exp(-x))


def _host_prep(inputs):
    """Embedding h0, pairwise d2, adjacency (all exact, as in the dense ref)."""
    x = np.concatenate([inputs["mol_x"], inputs["pocket_x"]], axis=1).astype(_F32)
    mask = np.concatenate([inputs["node_mask"], inputs["pocket_mask"]], axis=1).astype(
        _F32
    )
    hm = _np_silu(inputs["mol_h"].astype(_F32) @ inputs["W_mol"] + inputs["b_mol"])
    hp = _np_silu(
        inputs["pocket_h"].astype(_F32) @ inputs["W_pok"] + inputs["b_pok"]
    )
    h0 = (
        np.concatenate([hm, hp], axis=1) @ inputs["W_emb"] + inputs["b_emb"]
    ).astype(_F32)  # [B, N, H]

    diff = x[:, :, None, :] - x[:, None, :, :]
    d2 = np.sum(diff * diff, axis=-1, dtype=_F32)  # [B, N, N]
    idx = np.arange(N)
    lig_pair = (idx[:, None] < NS) & (idx[None, :] < NS)
    adj = np.where(lig_pair, 1.0, (d2 <= CUT2).astype(_F32))
    adj = adj * mask[:, :, None] * mask[:, None, :]
    return h0, d2, adj, mask


def _pack_edges(d2, adj, s_max):
    """Sort real edges by row, pad rows to SLOT multiples. Returns the one-hot
    col-gather blocks, slot->row gather P (for ha), the combined ones/d2
    pattern, adj logit row, and the slot->row combine matrix R."""
    import ml_dtypes

    bf = ml_dtypes.bfloat16
    ep = s_max * SLOT
    scol = np.zeros((2, 128, ep), dtype=bf)   # one-hot of j
    srow = np.zeros((2, 128, ep), dtype=bf)   # one-hot of i
    d2g = np.zeros((3, ep), dtype=bf)         # d2 rows [hi, lo, hi]
    adjg = np.full((1, ep), -1.0e9, dtype=bf)
    rmat = np.zeros((128, 3 * N), dtype=bf)   # R[s, i] in 3 K-blocks of 128

    s = 0
    for i in range(N):
        js = np.nonzero(adj[i] > 0)[0]
        nsl = (len(js) + SLOT - 1) // SLOT
        assert s + nsl <= s_max, "slot budget exceeded"
        for k, j in enumerate(js):
            e = s * SLOT + k
            scol[j // 128, j % 128, e] = 1.0
            srow[i // 128, i % 128, e] = 1.0
            v = _F32(d2[i, j])
            hi = bf(v)
            d2g[0, e] = hi
            d2g[1, e] = bf(v - _F32(hi))
            d2g[2, e] = hi
            adjg[0, e] = 0.0
        for q in range(nsl):
            sq = s + q
            rmat[sq % 128, (sq // 128) * N + i] = 1.0
        s += nsl
    return scol, srow, d2g, adjg, rmat


# weight-pack column offsets (per layer stride)
_PL = 6 * HID + 8  # wa, wb, We2, Wn1a, Wn1b, Wn2 (128 each) + small cols
_W_COLS = NLAYERS * _PL + HID + 8  # + W_out + wlin/bout/blin
_PLB = 2 * HID  # bf16 pack per layer: We2 | WatFull (Wat in all 128 cols)
_WB_COLS = NLAYERS * _PLB


def _pack_weights(inputs):
    import ml_dtypes

    bf = ml_dtypes.bfloat16
    wp = np.zeros((HID, _W_COLS), dtype=_F32)
    wcr3 = np.zeros((3, NLAYERS * HID), dtype=bf)
    wpb = np.zeros((HID, _WB_COLS), dtype=bf)
    We1 = inputs["We1"].astype(_F32)
    for l in range(NLAYERS):
        o = l * _PL
        wp[:, o : o + HID] = We1[l, :HID, :]              # wa
        wp[:, o + HID : o + 2 * HID] = We1[l, HID : 2 * HID, :]  # wb
        wp[:, o + 2 * HID : o + 3 * HID] = inputs["We2"][l]
        wp[:, o + 3 * HID : o + 4 * HID] = inputs["Wn1"][l][:HID, :]
        wp[:, o + 4 * HID : o + 5 * HID] = inputs["Wn1"][l][HID:, :]
        wp[:, o + 5 * HID : o + 6 * HID] = inputs["Wn2"][l]
        c = o + 6 * HID
        wp[:, c + 1] = inputs["be1"][l]
        wp[:, c + 2] = inputs["be2"][l]
        wp[:, c + 3] = inputs["bn1"][l]
        wp[:, c + 4] = inputs["bn2"][l]
        wp[:, c + 5] = inputs["bat"][l][0]                # bat replicated
        wc = We1[l, 2 * HID, :]
        hi = wc.astype(bf)
        wcr3[0, l * HID : (l + 1) * HID] = hi
        wcr3[1, l * HID : (l + 1) * HID] = hi
        wcr3[2, l * HID : (l + 1) * HID] = (wc - hi.astype(_F32)).astype(bf)
        ob = l * _PLB
        wpb[:, ob : ob + HID] = inputs["We2"][l]
        wpb[:, ob + HID : ob + 2 * HID] = np.repeat(
            inputs["Wat"][l].astype(_F32), HID, axis=1
        )
    o = NLAYERS * _PL
    wp[:, o : o + HID] = inputs["W_out"].astype(_F32)
    wp[:, o + HID] = inputs["W_lin"][:, 0]
    wp[:, o + HID + 1] = inputs["b_out"]
    wp[0, o + HID + 2] = inputs["b_lin"][0]
    return wp, wcr3, wpb


def _host_layer0(h0b, wp):
    """Precompute layer-0 habRb [128, 512] on host."""
    import ml_dtypes

    bf = ml_dtypes.bfloat16
    wa = wp[:, 0:HID]
    wb = wp[:, HID : 2 * HID]
    haR = (h0b @ wa).astype(bf)               # [N, H]
    hbR = (h0b @ wb).astype(bf)
    habrb = np.zeros((128, 4 * HID), dtype=bf)
    habrb[:, 0:HID] = haR[:128]
    habrb[:, HID : 2 * HID] = haR[128:]
    habrb[:, 2 * HID : 3 * HID] = hbR[:128]
    habrb[:, 3 * HID : 4 * HID] = hbR[128:]
    return habrb


def _build(nc, tile_mod, bass_mod, n_layers):
    """Trace the per-core kernel into nc (a Bacc)."""
    mybir = __import__("concourse.mybir", fromlist=["mybir"])
    dt = mybir.dt.float32
    bf = mybir.dt.bfloat16
    AF = mybir.ActivationFunctionType
    ALU = mybir.AluOpType

    hT_d = nc.dram_tensor("hT0", [HID, N], dt, kind="ExternalInput")
    srA_d = nc.dram_tensor("srowA", [128, EP], bf, kind="ExternalInput")
    srB_d = nc.dram_tensor("srowB", [128, EP], bf, kind="ExternalInput")
    scA_d = nc.dram_tensor("scolA", [128, EP], bf, kind="ExternalInput")
    scB_d = nc.dram_tensor("scolB", [128, EP], bf, kind="ExternalInput")
    d2_d = nc.dram_tensor("d2g", [3, EP], bf, kind="ExternalInput")
    aj_d = nc.dram_tensor("adjg", [1, EP], bf, kind="ExternalInput")
    rm_d = nc.dram_tensor("rmat", [128, 3 * N], bf, kind="ExternalInput")
    id_d = nc.dram_tensor("ident", [128, 128], dt, kind="ExternalInput")
    wp_d = nc.dram_tensor("wpack", [HID, _W_COLS], dt, kind="ExternalInput")
    wc_d = nc.dram_tensor("wcr3", [3, NLAYERS * HID], bf, kind="ExternalInput")
    wb_d = nc.dram_tensor("wpackb", [HID, _WB_COLS], bf, kind="ExternalInput")
    hb0_d = nc.dram_tensor("habrb0", [128, 4 * HID], bf, kind="ExternalInput")
    out_d = nc.dram_tensor("out", [1, NS], dt, kind="ExternalOutput")

    with tile_mod.TileContext(nc) as tc:
        with (
            tc.tile_pool(name="const", bufs=1) as cpool,
            tc.tile_pool(name="layer", bufs=2) as lpool,
            tc.tile_pool(name="work", bufs=4) as wpool,
            tc.tile_pool(name="psA", bufs=3, space="PSUM") as psA,
            tc.tile_pool(name="psB", bufs=2, space="PSUM") as psB,
            tc.tile_pool(name="psC", bufs=2, space="PSUM") as psC,
            tc.tile_pool(name="psD", bufs=1, space="PSUM") as psD,
        ):
            # ---- constants; DMA issue order = startup critical path ----
            hab0 = cpool.tile([128, 4 * HID], bf, tag="habrb0")
            d2g = cpool.tile([3, EP], bf, tag="d2g")
            adjg = cpool.tile([1, EP], bf, tag="adjg")
            wcr3 = cpool.tile([3, NLAYERS * HID], bf, tag="wcr3")
            srowA = cpool.tile([128, EP], bf, tag="srowA")
            srowB = cpool.tile([128, EP], bf, tag="srowB")
            scolA = cpool.tile([128, EP], bf, tag="scolA")
            scolB = cpool.tile([128, EP], bf, tag="scolB")
            wpb = cpool.tile([HID, _WB_COLS], bf, tag="wpackb")
            wp = cpool.tile([HID, _W_COLS], dt, tag="wpack")
            hT = cpool.tile([HID, N], dt, tag="hT0")
            rmat = cpool.tile([128, 3 * N], bf, tag="rmat")
            ident = cpool.tile([128, 128], dt, tag="ident")
            nc.sync.dma_start(hab0[:], hb0_d.ap())
            nc.sync.dma_start(d2g[:], d2_d.ap())
            nc.sync.dma_start(adjg[:], aj_d.ap())
            nc.sync.dma_start(wcr3[:], wc_d.ap())
            nc.sync.dma_start(wpb[:], wb_d.ap())
            # one-hot gathers: split loads so early tiles start sooner
            _NCH = 8
            chw = EP // _NCH
            for ch in range(_NCH):
                sl = slice(ch * chw, (ch + 1) * chw)
                nc.sync.dma_start(srowA[:, sl], srA_d.ap()[:, sl])
                nc.sync.dma_start(srowB[:, sl], srB_d.ap()[:, sl])
                nc.sync.dma_start(scolA[:, sl], scA_d.ap()[:, sl])
                nc.sync.dma_start(scolB[:, sl], scB_d.ap()[:, sl])
                if ch == 0:
                    nc.sync.dma_start(wp[:], wp_d.ap())
            nc.sync.dma_start(hT[:], hT_d.ap())
            nc.sync.dma_start(rmat[:], rm_d.ap())
            nc.sync.dma_start(ident[:], id_d.ap())
            ones1 = cpool.tile([1, HID], bf, tag="ones1")
            nc.vector.memset(ones1[:], 1.0)
            aggS = cpool.tile([HID, S_MAX], dt, tag="aggS")

            hT_cur = hT
            for l in range(n_layers):
                o = l * _PL
                Wn1a = wp[:, o + 3 * HID : o + 4 * HID]
                Wn1b = wp[:, o + 4 * HID : o + 5 * HID]
                Wn2 = wp[:, o + 5 * HID : o + 6 * HID]
                c = o + 6 * HID
                be1 = wp[:, c + 1 : c + 2]
                be2 = wp[:, c + 2 : c + 3]
                bn1 = wp[:, c + 3 : c + 4]
                bn2 = wp[:, c + 4 : c + 5]
                bat = wp[:, c + 5 : c + 6]
                ob = l * _PLB
                We2b = wpb[:, ob : ob + HID]
                WatF = wpb[:, ob + HID : ob + 2 * HID]

                if l == 0:
                    habRb = hab0
                else:
                    # node projections haR/hbR in [node, feat] layout
                    wa = wp[:, o : o + HID]
                    wb_ = wp[:, o + HID : o + 2 * HID]
                    ps_hab = psD.tile([128, 4 * HID], dt, tag="gath")
                    nc.tensor.matmul(ps_hab[:, 0:HID], hT_cur[:, 0:128], wa,
                                     start=True, stop=True)
                    nc.tensor.matmul(ps_hab[:, HID : 2 * HID],
                                     hT_cur[:, 128:256], wa,
                                     start=True, stop=True)
                    nc.tensor.matmul(ps_hab[:, 2 * HID : 3 * HID],
                                     hT_cur[:, 0:128], wb_,
                                     start=True, stop=True)
                    nc.tensor.matmul(ps_hab[:, 3 * HID : 4 * HID],
                                     hT_cur[:, 128:256], wb_,
                                     start=True, stop=True)
                    habRb = lpool.tile([128, 4 * HID], bf, tag="habRb")
                    nc.scalar.activation(habRb[:, 0 : 2 * HID],
                                         ps_hab[:, 0 : 2 * HID],
                                         AF.Identity, bias=0.0)
                    nc.scalar.activation(habRb[:, 2 * HID : 4 * HID],
                                         ps_hab[:, 2 * HID : 4 * HID],
                                         AF.Identity, bias=0.0)

                # ---- edge tiles, 5-stage skewed software pipeline ----
                # per-iteration emission order M, A, G, R, P keeps the pre
                # matmuls (which wait on psA rotation) at the back of the PE
                # queue, behind m1/att which have older, already-met deps
                st = {}   # in-flight per-tile tiles: st[t] = dict
                for it in range(NT + 5):
                    # stage M: edge MLP second layer
                    if 0 <= it - 2 < NT:
                        t = it - 2
                        ps_m1 = psB.tile([HID, TILE], dt, tag="m1")
                        nc.tensor.matmul(ps_m1[:], We2b, st[t]["rpre"][:],
                                         start=True, stop=True)
                        m = wpool.tile([HID, TILE], bf, tag="m", bufs=6)
                        if t % 3 == 0:
                            nc.vector.tensor_scalar(
                                m[:], ps_m1[:], be2, 0.0, ALU.add, ALU.max
                            )
                        else:
                            nc.scalar.activation(m[:], ps_m1[:], AF.Relu,
                                                 bias=be2)
                        st[t]["m"] = m
                    # stage A: attention + sigmoid
                    if 0 <= it - 3 < NT:
                        t = it - 3
                        sl = slice(t * TILE, (t + 1) * TILE)
                        ps_att = psC.tile([HID, TILE], dt, tag="att")
                        nc.tensor.matmul(ps_att[:], WatF, st[t]["m"][:],
                                         start=True, stop=False)
                        nc.tensor.matmul(ps_att[:], ones1[:], adjg[:, sl],
                                         start=False, stop=True)
                        sigp = wpool.tile([HID, TILE], bf, tag="sigp", bufs=5)
                        nc.scalar.activation(sigp[:], ps_att[:], AF.Sigmoid,
                                             bias=bat)
                        st[t]["sigp"] = sigp
                    # stage G: gated per-slot aggregation
                    if 0 <= it - 5 < NT:
                        t = it - 5
                        m, sigp = st[t]["m"], st[t]["sigp"]
                        mg = wpool.tile([HID, TILE], bf, tag="mg", bufs=2)
                        for k in range(SPT):
                            s = t * SPT + k
                            ksl = slice(k * SLOT, (k + 1) * SLOT)
                            nc.vector.scalar_tensor_tensor(
                                out=mg[:, ksl], in0=m[:, ksl],
                                scalar=1.0 / NORM, in1=sigp[:, ksl],
                                op0=ALU.mult, op1=ALU.mult,
                                accum_out=aggS[:, s : s + 1],
                            )
                        del st[t]
                        # transpose finished aggS blocks early
                        if t == 7 or t == 15:
                            q = t // 8
                            ps_t = psD.tile([128, 128], dt, tag="gath",
                                            name="ps_t")
                            nc.tensor.transpose(
                                ps_t[:], aggS[:, q * 128 : (q + 1) * 128],
                                ident[:])
                            if "aggSTb" not in st:
                                st["aggSTb"] = lpool.tile(
                                    [128, S_MAX], bf, tag="aggSTb",
                                    name="aggSTb")
                            aggSTb = st["aggSTb"]
                            nc.scalar.activation(
                                aggSTb[:, q * 128 : (q + 1) * 128], ps_t[:],
                                AF.Identity, bias=0.0)
                    # stage R: relu1
                    if 0 <= it - 1 < NT:
                        t = it - 1
                        rpre = wpool.tile([HID, TILE], bf, tag="rpre", bufs=4)
                        nc.scalar.activation(rpre[:], st[t]["pre"][:],
                                             AF.Relu, bias=be1)
                        st[t]["rpre"] = rpre
                    # stage P: pre accumulation for tile it
                    if it < NT:
                        t = it
                        sl = slice(t * TILE, (t + 1) * TILE)
                        ps_pre = psA.tile([HID, TILE], dt, tag="pre")
                        nc.tensor.matmul(ps_pre[:], habRb[:, 0:HID],
                                         srowA[:, sl], start=True, stop=False)
                        nc.tensor.matmul(ps_pre[:], habRb[:, HID : 2 * HID],
                                         srowB[:, sl], start=False, stop=False)
                        nc.tensor.matmul(ps_pre[:], habRb[:, 2 * HID : 3 * HID],
                                         scolA[:, sl], start=False, stop=False)
                        nc.tensor.matmul(ps_pre[:], habRb[:, 3 * HID : 4 * HID],
                                         scolB[:, sl], start=False, stop=False)
                        nc.tensor.matmul(ps_pre[:],
                                         wcr3[:, l * HID : (l + 1) * HID],
                                         d2g[:, sl], start=False, stop=True)
                        st[t] = {"pre": ps_pre}

                # ---- last aggS block -> rows, node MLP ----
                aggSTb = st["aggSTb"]
                ps_t2 = psD.tile([128, 128], dt, tag="gath", name="ps_t2")
                nc.tensor.transpose(ps_t2[:], aggS[:, 256:384], ident[:])
                nc.scalar.activation(aggSTb[:, 256:384], ps_t2[:],
                                     AF.Identity, bias=0.0)
                ps_agg = psB.tile([HID, N], dt, tag="m1")
                for q in range(3):
                    nc.tensor.matmul(
                        ps_agg[:], aggSTb[:, q * 128 : (q + 1) * 128],
                        rmat[:, q * N : (q + 1) * N],
                        start=(q == 0), stop=(q == 2),
                    )
                aggT = lpool.tile([HID, N], dt, tag="aggT")
                nc.vector.tensor_copy(aggT[:], ps_agg[:])

                ps_n1 = psA.tile([HID, N], dt, tag="pre")
                nc.tensor.matmul(ps_n1[:], Wn1a, hT_cur[:], start=True, stop=False)
                nc.tensor.matmul(ps_n1[:], Wn1b, aggT[:], start=False, stop=True)
                t1 = wpool.tile([HID, N], dt, tag="nodet")
                nc.scalar.activation(t1[:], ps_n1[:], AF.Relu, bias=bn1)
                ps_n2 = psB.tile([HID, N], dt, tag="m1")
                nc.tensor.matmul(ps_n2[:], Wn2, t1[:], start=True, stop=True)
                hT_new = lpool.tile([HID, N], dt, tag="hT")
                nc.vector.scalar_tensor_tensor(
                    out=hT_new[:], in0=ps_n2[:], scalar=bn2, in1=hT_cur[:],
                    op0=ALU.add, op1=ALU.add,
                )
                hT_cur = hT_new

            # ---- output head ----
            o = NLAYERS * _PL
            W_out = wp[:, o : o + HID]
            W_lin = wp[:, o + HID : o + HID + 1]
            b_out = wp[:, o + HID + 1 : o + HID + 2]
            b_lin = wp[0:1, o + HID + 2 : o + HID + 3]
            ps_o = psA.tile([HID, NS], dt, tag="pre")
            nc.tensor.matmul(ps_o[:], W_out, hT_cur[:, 0:NS], start=True, stop=True)
            ho = wpool.tile([HID, NS], dt, tag="nodet")
            nc.scalar.activation(ho[:], ps_o[:], AF.Relu, bias=b_out)
            ps_y = psB.tile([1, NS], dt, tag="m1")
            nc.tensor.matmul(ps_y[:], W_lin, ho[:], start=True, stop=True)
            y = wpool.tile([1, NS], dt, tag="ytile")
            nc.scalar.activation(y[:], ps_y[:], AF.Identity, bias=b_lin)
            nc.sync.dma_start(out_d.ap(), y[:])


def _make_in_maps(inputs, n_layers):
    h0, d2, adj, mask = _host_prep(inputs)
    wp, wcr3, wpb = _pack_weights(inputs)
    ident = np.eye(128, dtype=_F32)
    in_maps = []
    for b in range(B):
        scol, srow, d2g, adjg, rmat = _pack_edges(d2[b], adj[b], S_MAX)
        habrb0 = _host_layer0(h0[b], wp)
        in_maps.append(
            {
                "hT0": np.ascontiguousarray(h0[b].T),
                "srowA": srow[0], "srowB": srow[1],
                "scolA": scol[0], "scolB": scol[1],
                "d2g": d2g, "adjg": adjg, "rmat": rmat, "ident": ident,
                "wpack": wp, "wcr3": wcr3, "wpackb": wpb,
                "habrb0": habrb0,
            }
        )
    return in_maps, mask


def _install_ntff_hook():
    """Recreate the antenv.axon_hooks module the boot expected, register the
    ctypes NTFF hook from trn_agent_boot, so run_bass_kernel_spmd(trace=True)
    can capture hardware profiles under axon."""
    import types

    if "antenv.axon_hooks" not in sys.modules:
        mod = types.ModuleType("antenv.axon_hooks")
        holder = [None]
        mod.set_axon_ntff_profile_hook = lambda h: holder.__setitem__(0, h)
        mod.get_axon_ntff_profile_hook = lambda: holder[0]
        sys.modules["antenv.axon_hooks"] = mod
        import antenv

        antenv.axon_hooks = mod
    m = sys.modules["antenv.axon_hooks"]
    if m.get_axon_ntff_profile_hook() is None:
        sys.path.insert(0, "/root/.axon_site")
        from trn_agent_boot.trn_boot import _ntff_profile_via_ctypes

        m.set_axon_ntff_profile_hook(
            _ntff_profile_via_ctypes("/opt/axon/libaxon_pjrt.so")
        )


_CACHE = {}


def _get_nc(n_layers):
    key = n_layers
    if key not in _CACHE:
        import concourse.bass as bass
        import concourse.tile as tile
        from concourse import bacc

        nc = bacc.Bacc(
            "TRN2", target_bir_lowering=False, debug=False, num_devices=B
        )
        _build(nc, tile, bass, n_layers)
        nc.compile()
        _CACHE[key] = nc
    return _CACHE[key]


def kernel(**inputs):
    inputs = {k: np.asarray(v) for k, v in inputs.items()}
    n_layers = int(os.environ.get("GNN_LAYERS", NLAYERS))
    in_maps, mask = _make_in_maps(inputs, n_layers)
    nc = _get_nc(n_layers)

    if os.environ.get("GNN_SIM"):
        from concourse.bass_interp import CoreSim

        outs = []
        for b in range(int(os.environ.get("GNN_SIM_CORES", 1))):
            sim = CoreSim(nc, trace=False)
            for k, v in in_maps[b].items():
                sim.tensor(k)[:] = v
            sim.simulate()
            outs.append(np.array(sim.tensor("out")).reshape(NS, 1))
        while len(outs) < B:
            outs.append(np.zeros((NS, 1), _F32))
        out = np.stack(outs)
    else:
        from concourse.bass_utils import run_bass_kernel_spmd

        if os.environ.get("GNN_TRACE"):
            _install_ntff_hook()
            tmpdir = os.environ.get("GNN_TRACE_DIR") or None
            try:
                res = run_bass_kernel_spmd(
                    nc, in_maps, core_ids=list(range(B)), trace=True, tmpdir=tmpdir
                )
                kernel.last_exec_time_ns = res.exec_time_ns
            except Exception as e:
                print(f"[gnn] traced run failed ({e!r}); retrying untraced")
                res = run_bass_kernel_spmd(nc, in_maps, core_ids=list(range(B)))
        else:
            res = run_bass_kernel_spmd(nc, in_maps, core_ids=list(range(B)))
        kernel.last_results = res
        out = np.stack([r["out"].reshape(NS, 1) for r in res.results])

    return (out * inputs["node_mask"][:, :, None]).astype(_F32)


# revision 50
# speedup vs baseline: 1.1472x; 1.1472x over previous
"""Trainium2 Bass kernel for AnchorGNNPocket (GNN message passing), sparse.

Data-parallel over batch B=8: one complex per NeuronCore. The cutoff graph is
only ~12% dense (7.3k-8.7k real edges of 65536 pairs), so instead of the dense
[N,N,H] edge sweep, each sample's edges are sorted by destination row i and
padded per-row to a multiple of 32 ("slots"); all samples pad to a fixed
S_MAX=384 slots (EP=12288 edge columns) so every core runs the identical
instruction stream and only tensor *contents* differ per sample.

Per [128, 512] edge tile (16 slots), the stages are
  pre  = gather(ha,row) + gather(hb,col) + wc (x) d2 + be1
         -- ONE PSUM group: 4x K=128 one-hot row/col gather matmuls (bf16)
         + the K=3 bf16 hi/lo d2 split; every matmul keeps lhsT at
         partition base 0 / PE quadrant q0 (mixed-quadrant lhsT tiles and
         mid-kernel SBUF->SBUF DMAs both measurably left the PE clock
         HAM-throttled at 1.2 GHz -- tried and reverted)
  m    = relu(We2^T relu(pre+be1) + be2)
  att  = WatFull^T m + adj_logit (K=1) + bat
  sigp = sigmoid(att)        (pad cols carry -1e9 logits -> sigp == 0)
  aggS[:, s] = sum_e (m/NORM)*sigp per slot   (DVE stst accum_out)
and the five stages are emitted SKEWED (pre(t), relu1(t-1), m(t-2), att(t-3),
gating(t-5)) so every engine always has independent work in flight; the
m-relu alternates DVE (1/3) and ScalarE (2/3) to balance engine load.

Slot sums combine to rows via PE transposes of aggS (blocks 0/1 overlapped
with the edge loop) + a one-hot R[s->i] matmul at layer end; layer-0 node
projections ha/hb are precomputed on host to shorten the startup critical
path, and the big one-hot gather matrices stream in 8 DMA chunks so tile 0
can start after the first chunk lands.
"""

import os
import sys

import numpy as np

if not any(os.path.isdir(os.path.join(p, "concourse")) for p in sys.path if p):
    sys.path.insert(0, "/opt/trn_rl_repo")

# ---- problem constants (hardcoded per contest rules) ----
B, NS, NP = 8, 32, 224
N = NS + NP                      # 256 nodes
LIG_NF, POK_NF, JNF, HID, OUT_NF, NLAYERS = 10, 25, 32, 128, 128, 4
CUT2 = 4.5 ** 2
NORM = 100.0

_F32 = np.float32
SLOT = 32                        # edge columns per slot (one row per slot)
S_MAX = 384                      # padded slot count (shared by all cores)
EP = S_MAX * SLOT                # 12288 edge columns
TILE = 512                       # edge columns per PSUM tile
SPT = TILE // SLOT               # 16 slots per tile
NT = EP // TILE                  # 24 tiles
LNT = 6                          # last-layer tiles: only ligand-row slots
                                 # (output head reads h[:, :NS] only)


def _np_silu(x):
    return x / (1.0 + np.exp(-x))


def _host_prep(inputs):
    """Embedding h0, pairwise d2, adjacency (all exact, as in the dense ref)."""
    x = np.concatenate([inputs["mol_x"], inputs["pocket_x"]], axis=1).astype(_F32)
    mask = np.concatenate([inputs["node_mask"], inputs["pocket_mask"]], axis=1).astype(
        _F32
    )
    hm = _np_silu(inputs["mol_h"].astype(_F32) @ inputs["W_mol"] + inputs["b_mol"])
    hp = _np_silu(
        inputs["pocket_h"].astype(_F32) @ inputs["W_pok"] + inputs["b_pok"]
    )
    h0 = (
        np.concatenate([hm, hp], axis=1) @ inputs["W_emb"] + inputs["b_emb"]
    ).astype(_F32)  # [B, N, H]

    diff = x[:, :, None, :] - x[:, None, :, :]
    d2 = np.sum(diff * diff, axis=-1, dtype=_F32)  # [B, N, N]
    idx = np.arange(N)
    lig_pair = (idx[:, None] < NS) & (idx[None, :] < NS)
    adj = np.where(lig_pair, 1.0, (d2 <= CUT2).astype(_F32))
    adj = adj * mask[:, :, None] * mask[:, None, :]
    return h0, d2, adj, mask


def _pack_edges(d2, adj, s_max):
    """Sort real edges by row, pad rows to SLOT multiples. Returns the one-hot
    col-gather blocks, slot->row gather P (for ha), the combined ones/d2
    pattern, adj logit row, and the slot->row combine matrix R."""
    import ml_dtypes

    bf = ml_dtypes.bfloat16
    ep = s_max * SLOT
    scol = np.zeros((2, 128, ep), dtype=bf)   # one-hot of j
    srow = np.zeros((2, 128, ep), dtype=bf)   # one-hot of i
    d2g = np.zeros((3, ep), dtype=bf)         # d2 rows [hi, lo, hi]
    adjg = np.full((1, ep), -1.0e9, dtype=bf)
    rmat = np.zeros((128, 3 * N), dtype=bf)   # R[s, i] in 3 K-blocks of 128

    s = 0
    for i in range(N):
        js = np.nonzero(adj[i] > 0)[0]
        nsl = (len(js) + SLOT - 1) // SLOT
        assert s + nsl <= s_max, "slot budget exceeded"
        if i == NS - 1:
            pass  # checked after loop via closure below
        for k, j in enumerate(js):
            e = s * SLOT + k
            scol[j // 128, j % 128, e] = 1.0
            srow[i // 128, i % 128, e] = 1.0
            v = _F32(d2[i, j])
            hi = bf(v)
            d2g[0, e] = hi
            d2g[1, e] = bf(v - _F32(hi))
            d2g[2, e] = hi
            adjg[0, e] = 0.0
        for q in range(nsl):
            sq = s + q
            rmat[sq % 128, (sq // 128) * N + i] = 1.0
        s += nsl
        if i == NS - 1:
            assert s <= LNT * SPT, "ligand slots exceed last-layer tiles"
    return scol, srow, d2g, adjg, rmat


# weight-pack column offsets (per layer stride)
_PL = 6 * HID + 8  # wa, wb, We2, Wn1a, Wn1b, Wn2 (128 each) + small cols
_W_COLS = NLAYERS * _PL + HID + 8  # + W_out + wlin/bout/blin
_PLB = 2 * HID  # bf16 pack per layer: We2 | WatFull (Wat in all 128 cols)
_WB_COLS = NLAYERS * _PLB


def _pack_weights(inputs):
    import ml_dtypes

    bf = ml_dtypes.bfloat16
    wp = np.zeros((HID, _W_COLS), dtype=_F32)
    wcr3 = np.zeros((3, NLAYERS * HID), dtype=bf)
    wpb = np.zeros((HID, _WB_COLS), dtype=bf)
    We1 = inputs["We1"].astype(_F32)
    for l in range(NLAYERS):
        o = l * _PL
        wp[:, o : o + HID] = We1[l, :HID, :]              # wa
        wp[:, o + HID : o + 2 * HID] = We1[l, HID : 2 * HID, :]  # wb
        wp[:, o + 2 * HID : o + 3 * HID] = inputs["We2"][l]
        wp[:, o + 3 * HID : o + 4 * HID] = inputs["Wn1"][l][:HID, :]
        wp[:, o + 4 * HID : o + 5 * HID] = inputs["Wn1"][l][HID:, :]
        wp[:, o + 5 * HID : o + 6 * HID] = inputs["Wn2"][l]
        c = o + 6 * HID
        wp[:, c + 1] = inputs["be1"][l]
        wp[:, c + 2] = inputs["be2"][l]
        wp[:, c + 3] = inputs["bn1"][l]
        wp[:, c + 4] = inputs["bn2"][l]
        wp[:, c + 5] = inputs["bat"][l][0]                # bat replicated
        wc = We1[l, 2 * HID, :]
        hi = wc.astype(bf)
        wcr3[0, l * HID : (l + 1) * HID] = hi
        wcr3[1, l * HID : (l + 1) * HID] = hi
        wcr3[2, l * HID : (l + 1) * HID] = (wc - hi.astype(_F32)).astype(bf)
        ob = l * _PLB
        wpb[:, ob : ob + HID] = inputs["We2"][l]
        wpb[:, ob + HID : ob + 2 * HID] = np.repeat(
            inputs["Wat"][l].astype(_F32), HID, axis=1
        )
    o = NLAYERS * _PL
    wp[:, o : o + HID] = inputs["W_out"].astype(_F32)
    wp[:, o + HID] = inputs["W_lin"][:, 0]
    wp[:, o + HID + 1] = inputs["b_out"]
    wp[0, o + HID + 2] = inputs["b_lin"][0]
    return wp, wcr3, wpb


def _host_layer0(h0b, wp):
    """Precompute layer-0 habRb [128, 512] on host."""
    import ml_dtypes

    bf = ml_dtypes.bfloat16
    wa = wp[:, 0:HID]
    wb = wp[:, HID : 2 * HID]
    haR = (h0b @ wa).astype(bf)               # [N, H]
    hbR = (h0b @ wb).astype(bf)
    habrb = np.zeros((128, 4 * HID), dtype=bf)
    habrb[:, 0:HID] = haR[:128]
    habrb[:, HID : 2 * HID] = haR[128:]
    habrb[:, 2 * HID : 3 * HID] = hbR[:128]
    habrb[:, 3 * HID : 4 * HID] = hbR[128:]
    return habrb


def _build(nc, tile_mod, bass_mod, n_layers):
    """Trace the per-core kernel into nc (a Bacc)."""
    mybir = __import__("concourse.mybir", fromlist=["mybir"])
    dt = mybir.dt.float32
    bf = mybir.dt.bfloat16
    AF = mybir.ActivationFunctionType
    ALU = mybir.AluOpType

    hT_d = nc.dram_tensor("hT0", [HID, N], dt, kind="ExternalInput")
    srA_d = nc.dram_tensor("srowA", [128, EP], bf, kind="ExternalInput")
    srB_d = nc.dram_tensor("srowB", [128, EP], bf, kind="ExternalInput")
    scA_d = nc.dram_tensor("scolA", [128, EP], bf, kind="ExternalInput")
    scB_d = nc.dram_tensor("scolB", [128, EP], bf, kind="ExternalInput")
    d2_d = nc.dram_tensor("d2g", [3, EP], bf, kind="ExternalInput")
    aj_d = nc.dram_tensor("adjg", [1, EP], bf, kind="ExternalInput")
    rm_d = nc.dram_tensor("rmat", [128, 3 * N], bf, kind="ExternalInput")
    id_d = nc.dram_tensor("ident", [128, 128], dt, kind="ExternalInput")
    wp_d = nc.dram_tensor("wpack", [HID, _W_COLS], dt, kind="ExternalInput")
    wc_d = nc.dram_tensor("wcr3", [3, NLAYERS * HID], bf, kind="ExternalInput")
    wb_d = nc.dram_tensor("wpackb", [HID, _WB_COLS], bf, kind="ExternalInput")
    hb0_d = nc.dram_tensor("habrb0", [128, 4 * HID], bf, kind="ExternalInput")
    out_d = nc.dram_tensor("out", [1, NS], dt, kind="ExternalOutput")

    with tile_mod.TileContext(nc) as tc:
        with (
            tc.tile_pool(name="const", bufs=1) as cpool,
            tc.tile_pool(name="layer", bufs=2) as lpool,
            tc.tile_pool(name="work", bufs=4) as wpool,
            tc.tile_pool(name="psA", bufs=3, space="PSUM") as psA,
            tc.tile_pool(name="psB", bufs=2, space="PSUM") as psB,
            tc.tile_pool(name="psC", bufs=2, space="PSUM") as psC,
            tc.tile_pool(name="psD", bufs=1, space="PSUM") as psD,
        ):
            # ---- constants; DMA issue order = startup critical path ----
            hab0 = cpool.tile([128, 4 * HID], bf, tag="habrb0")
            d2g = cpool.tile([3, EP], bf, tag="d2g")
            adjg = cpool.tile([1, EP], bf, tag="adjg")
            wcr3 = cpool.tile([3, NLAYERS * HID], bf, tag="wcr3")
            srowA = cpool.tile([128, EP], bf, tag="srowA")
            srowB = cpool.tile([128, EP], bf, tag="srowB")
            scolA = cpool.tile([128, EP], bf, tag="scolA")
            scolB = cpool.tile([128, EP], bf, tag="scolB")
            wpb = cpool.tile([HID, _WB_COLS], bf, tag="wpackb")
            wp = cpool.tile([HID, _W_COLS], dt, tag="wpack")
            hT = cpool.tile([HID, N], dt, tag="hT0")
            rmat = cpool.tile([128, 3 * N], bf, tag="rmat")
            ident = cpool.tile([128, 128], dt, tag="ident")
            nc.sync.dma_start(hab0[:], hb0_d.ap())
            nc.sync.dma_start(d2g[:], d2_d.ap())
            nc.sync.dma_start(adjg[:], aj_d.ap())
            nc.sync.dma_start(wcr3[:], wc_d.ap())
            nc.sync.dma_start(wpb[:], wb_d.ap())
            # one-hot gathers: split loads so early tiles start sooner
            _NCH = 8
            chw = EP // _NCH
            for ch in range(_NCH):
                sl = slice(ch * chw, (ch + 1) * chw)
                nc.sync.dma_start(srowA[:, sl], srA_d.ap()[:, sl])
                nc.sync.dma_start(srowB[:, sl], srB_d.ap()[:, sl])
                nc.sync.dma_start(scolA[:, sl], scA_d.ap()[:, sl])
                nc.sync.dma_start(scolB[:, sl], scB_d.ap()[:, sl])
                if ch == 0:
                    nc.sync.dma_start(wp[:], wp_d.ap())
            nc.sync.dma_start(hT[:], hT_d.ap())
            nc.sync.dma_start(rmat[:], rm_d.ap())
            nc.sync.dma_start(ident[:], id_d.ap())
            ones1 = cpool.tile([1, HID], bf, tag="ones1")
            nc.vector.memset(ones1[:], 1.0)
            aggS = cpool.tile([HID, S_MAX], dt, tag="aggS")

            hT_cur = hT
            for l in range(n_layers):
                o = l * _PL
                Wn1a = wp[:, o + 3 * HID : o + 4 * HID]
                Wn1b = wp[:, o + 4 * HID : o + 5 * HID]
                Wn2 = wp[:, o + 5 * HID : o + 6 * HID]
                c = o + 6 * HID
                be1 = wp[:, c + 1 : c + 2]
                be2 = wp[:, c + 2 : c + 3]
                bn1 = wp[:, c + 3 : c + 4]
                bn2 = wp[:, c + 4 : c + 5]
                bat = wp[:, c + 5 : c + 6]
                ob = l * _PLB
                We2b = wpb[:, ob : ob + HID]
                WatF = wpb[:, ob + HID : ob + 2 * HID]

                nt_l = LNT if l == n_layers - 1 else NT
                if l == 0:
                    habRb = hab0
                else:
                    # node projections haR/hbR in [node, feat] layout
                    wa = wp[:, o : o + HID]
                    wb_ = wp[:, o + HID : o + 2 * HID]
                    ps_hab = psD.tile([128, 4 * HID], dt, tag="gath")
                    nc.tensor.matmul(ps_hab[:, 0:HID], hT_cur[:, 0:128], wa,
                                     start=True, stop=True)
                    nc.tensor.matmul(ps_hab[:, HID : 2 * HID],
                                     hT_cur[:, 128:256], wa,
                                     start=True, stop=True)
                    nc.tensor.matmul(ps_hab[:, 2 * HID : 3 * HID],
                                     hT_cur[:, 0:128], wb_,
                                     start=True, stop=True)
                    nc.tensor.matmul(ps_hab[:, 3 * HID : 4 * HID],
                                     hT_cur[:, 128:256], wb_,
                                     start=True, stop=True)
                    habRb = lpool.tile([128, 4 * HID], bf, tag="habRb")
                    nc.scalar.activation(habRb[:, 0 : 2 * HID],
                                         ps_hab[:, 0 : 2 * HID],
                                         AF.Identity, bias=0.0)
                    nc.scalar.activation(habRb[:, 2 * HID : 4 * HID],
                                         ps_hab[:, 2 * HID : 4 * HID],
                                         AF.Identity, bias=0.0)

                # ---- edge tiles, 5-stage skewed software pipeline ----
                # per-iteration emission order M, A, G, R, P keeps the pre
                # matmuls (which wait on psA rotation) at the back of the PE
                # queue, behind m1/att which have older, already-met deps
                st = {}   # in-flight per-tile tiles: st[t] = dict
                for it in range(nt_l + 5):
                    # stage M: edge MLP second layer
                    if 0 <= it - 2 < nt_l:
                        t = it - 2
                        ps_m1 = psB.tile([HID, TILE], dt, tag="m1")
                        nc.tensor.matmul(ps_m1[:], We2b, st[t]["rpre"][:],
                                         start=True, stop=True)
                        m = wpool.tile([HID, TILE], bf, tag="m", bufs=6)
                        if t % 3 == 0:
                            nc.vector.tensor_scalar(
                                m[:], ps_m1[:], be2, 0.0, ALU.add, ALU.max
                            )
                        else:
                            nc.scalar.activation(m[:], ps_m1[:], AF.Relu,
                                                 bias=be2)
                        st[t]["m"] = m
                    # stage A: attention + sigmoid
                    if 0 <= it - 3 < nt_l:
                        t = it - 3
                        sl = slice(t * TILE, (t + 1) * TILE)
                        ps_att = psC.tile([HID, TILE], dt, tag="att")
                        nc.tensor.matmul(ps_att[:], WatF, st[t]["m"][:],
                                         start=True, stop=False)
                        nc.tensor.matmul(ps_att[:], ones1[:], adjg[:, sl],
                                         start=False, stop=True)
                        sigp = wpool.tile([HID, TILE], bf, tag="sigp", bufs=5)
                        nc.scalar.activation(sigp[:], ps_att[:], AF.Sigmoid,
                                             bias=bat)
                        st[t]["sigp"] = sigp
                    # stage G: gated per-slot aggregation
                    if 0 <= it - 5 < nt_l:
                        t = it - 5
                        m, sigp = st[t]["m"], st[t]["sigp"]
                        mg = wpool.tile([HID, TILE], bf, tag="mg", bufs=2)
                        for k in range(SPT):
                            s = t * SPT + k
                            ksl = slice(k * SLOT, (k + 1) * SLOT)
                            nc.vector.scalar_tensor_tensor(
                                out=mg[:, ksl], in0=m[:, ksl],
                                scalar=1.0 / NORM, in1=sigp[:, ksl],
                                op0=ALU.mult, op1=ALU.mult,
                                accum_out=aggS[:, s : s + 1],
                            )
                        del st[t]
                        # transpose finished aggS blocks early
                        if nt_l == NT and (t == 7 or t == 15):
                            q = t // 8
                            ps_t = psD.tile([128, 128], dt, tag="gath",
                                            name="ps_t")
                            nc.tensor.transpose(
                                ps_t[:], aggS[:, q * 128 : (q + 1) * 128],
                                ident[:])
                            if "aggSTb" not in st:
                                st["aggSTb"] = lpool.tile(
                                    [128, S_MAX], bf, tag="aggSTb",
                                    name="aggSTb")
                            aggSTb = st["aggSTb"]
                            nc.scalar.activation(
                                aggSTb[:, q * 128 : (q + 1) * 128], ps_t[:],
                                AF.Identity, bias=0.0)
                    # stage R: relu1
                    if 0 <= it - 1 < nt_l:
                        t = it - 1
                        rpre = wpool.tile([HID, TILE], bf, tag="rpre", bufs=4)
                        nc.scalar.activation(rpre[:], st[t]["pre"][:],
                                             AF.Relu, bias=be1)
                        st[t]["rpre"] = rpre
                    # stage P: pre accumulation for tile it
                    if it < nt_l:
                        t = it
                        sl = slice(t * TILE, (t + 1) * TILE)
                        ps_pre = psA.tile([HID, TILE], dt, tag="pre")
                        nc.tensor.matmul(ps_pre[:], habRb[:, 0:HID],
                                         srowA[:, sl], start=True, stop=False)
                        nc.tensor.matmul(ps_pre[:], habRb[:, HID : 2 * HID],
                                         srowB[:, sl], start=False, stop=False)
                        nc.tensor.matmul(ps_pre[:], habRb[:, 2 * HID : 3 * HID],
                                         scolA[:, sl], start=False, stop=False)
                        nc.tensor.matmul(ps_pre[:], habRb[:, 3 * HID : 4 * HID],
                                         scolB[:, sl], start=False, stop=False)
                        nc.tensor.matmul(ps_pre[:],
                                         wcr3[:, l * HID : (l + 1) * HID],
                                         d2g[:, sl], start=False, stop=True)
                        st[t] = {"pre": ps_pre}

                if nt_l == NT:
                    # ---- last aggS block -> rows, node MLP (full layer) ----
                    aggSTb = st["aggSTb"]
                    ps_t2 = psD.tile([128, 128], dt, tag="gath", name="ps_t2")
                    nc.tensor.transpose(ps_t2[:], aggS[:, 256:384], ident[:])
                    nc.scalar.activation(aggSTb[:, 256:384], ps_t2[:],
                                         AF.Identity, bias=0.0)
                    ps_agg = psB.tile([HID, N], dt, tag="m1")
                    for q in range(3):
                        nc.tensor.matmul(
                            ps_agg[:], aggSTb[:, q * 128 : (q + 1) * 128],
                            rmat[:, q * N : (q + 1) * N],
                            start=(q == 0), stop=(q == 2),
                        )
                    aggT = lpool.tile([HID, N], dt, tag="aggT")
                    nc.vector.tensor_copy(aggT[:], ps_agg[:])

                    ps_n1 = psA.tile([HID, N], dt, tag="pre")
                    nc.tensor.matmul(ps_n1[:], Wn1a, hT_cur[:],
                                     start=True, stop=False)
                    nc.tensor.matmul(ps_n1[:], Wn1b, aggT[:],
                                     start=False, stop=True)
                    t1 = wpool.tile([HID, N], dt, tag="nodet")
                    nc.scalar.activation(t1[:], ps_n1[:], AF.Relu, bias=bn1)
                    ps_n2 = psB.tile([HID, N], dt, tag="m1")
                    nc.tensor.matmul(ps_n2[:], Wn2, t1[:], start=True, stop=True)
                    hT_new = lpool.tile([HID, N], dt, tag="hT")
                    nc.vector.scalar_tensor_tensor(
                        out=hT_new[:], in0=ps_n2[:], scalar=bn2, in1=hT_cur[:],
                        op0=ALU.add, op1=ALU.add,
                    )
                    hT_cur = hT_new
                else:
                    # ---- short last layer: ligand rows only. Slots of rows
                    # >= NS inside tiles 0..LNT-1 are dropped because the R
                    # rhs is sliced to columns 0:NS ----
                    ps_tS = psD.tile([128, 128], dt, tag="gath", name="ps_tS")
                    nc.tensor.transpose(ps_tS[:], aggS[:, 0:128], ident[:])
                    aggSTbS = lpool.tile([128, 128], bf, tag="aggSTbS")
                    nc.scalar.activation(aggSTbS[:], ps_tS[:],
                                         AF.Identity, bias=0.0)
                    ps_agg = psB.tile([HID, NS], dt, tag="m1")
                    nc.tensor.matmul(ps_agg[:], aggSTbS[:], rmat[:, 0:NS],
                                     start=True, stop=True)
                    aggT = lpool.tile([HID, NS], dt, tag="aggT")
                    nc.vector.tensor_copy(aggT[:], ps_agg[:])

                    ps_n1 = psA.tile([HID, NS], dt, tag="pre")
                    nc.tensor.matmul(ps_n1[:], Wn1a, hT_cur[:, 0:NS],
                                     start=True, stop=False)
                    nc.tensor.matmul(ps_n1[:], Wn1b, aggT[:],
                                     start=False, stop=True)
                    t1 = wpool.tile([HID, NS], dt, tag="nodet")
                    nc.scalar.activation(t1[:], ps_n1[:], AF.Relu, bias=bn1)
                    ps_n2 = psB.tile([HID, NS], dt, tag="m1")
                    nc.tensor.matmul(ps_n2[:], Wn2, t1[:], start=True, stop=True)
                    hT_new = lpool.tile([HID, NS], dt, tag="hT")
                    nc.vector.scalar_tensor_tensor(
                        out=hT_new[:], in0=ps_n2[:], scalar=bn2,
                        in1=hT_cur[:, 0:NS], op0=ALU.add, op1=ALU.add,
                    )
                    hT_cur = hT_new

            # ---- output head ----
            o = NLAYERS * _PL
            W_out = wp[:, o : o + HID]
            W_lin = wp[:, o + HID : o + HID + 1]
            b_out = wp[:, o + HID + 1 : o + HID + 2]
            b_lin = wp[0:1, o + HID + 2 : o + HID + 3]
            ps_o = psA.tile([HID, NS], dt, tag="pre")
            nc.tensor.matmul(ps_o[:], W_out, hT_cur[:, 0:NS], start=True, stop=True)
            ho = wpool.tile([HID, NS], dt, tag="nodet")
            nc.scalar.activation(ho[:], ps_o[:], AF.Relu, bias=b_out)
            ps_y = psB.tile([1, NS], dt, tag="m1")
            nc.tensor.matmul(ps_y[:], W_lin, ho[:], start=True, stop=True)
            y = wpool.tile([1, NS], dt, tag="ytile")
            nc.scalar.activation(y[:], ps_y[:], AF.Identity, bias=b_lin)
            nc.sync.dma_start(out_d.ap(), y[:])


def _make_in_maps(inputs, n_layers):
    h0, d2, adj, mask = _host_prep(inputs)
    wp, wcr3, wpb = _pack_weights(inputs)
    ident = np.eye(128, dtype=_F32)
    in_maps = []
    for b in range(B):
        scol, srow, d2g, adjg, rmat = _pack_edges(d2[b], adj[b], S_MAX)
        habrb0 = _host_layer0(h0[b], wp)
        in_maps.append(
            {
                "hT0": np.ascontiguousarray(h0[b].T),
                "srowA": srow[0], "srowB": srow[1],
                "scolA": scol[0], "scolB": scol[1],
                "d2g": d2g, "adjg": adjg, "rmat": rmat, "ident": ident,
                "wpack": wp, "wcr3": wcr3, "wpackb": wpb,
                "habrb0": habrb0,
            }
        )
    return in_maps, mask


def _install_ntff_hook():
    """Recreate the antenv.axon_hooks module the boot expected, register the
    ctypes NTFF hook from trn_agent_boot, so run_bass_kernel_spmd(trace=True)
    can capture hardware profiles under axon."""
    import types

    if "antenv.axon_hooks" not in sys.modules:
        mod = types.ModuleType("antenv.axon_hooks")
        holder = [None]
        mod.set_axon_ntff_profile_hook = lambda h: holder.__setitem__(0, h)
        mod.get_axon_ntff_profile_hook = lambda: holder[0]
        sys.modules["antenv.axon_hooks"] = mod
        import antenv

        antenv.axon_hooks = mod
    m = sys.modules["antenv.axon_hooks"]
    if m.get_axon_ntff_profile_hook() is None:
        sys.path.insert(0, "/root/.axon_site")
        from trn_agent_boot.trn_boot import _ntff_profile_via_ctypes

        m.set_axon_ntff_profile_hook(
            _ntff_profile_via_ctypes("/opt/axon/libaxon_pjrt.so")
        )


_CACHE = {}


def _get_nc(n_layers):
    key = n_layers
    if key not in _CACHE:
        import concourse.bass as bass
        import concourse.tile as tile
        from concourse import bacc

        nc = bacc.Bacc(
            "TRN2", target_bir_lowering=False, debug=False, num_devices=B
        )
        _build(nc, tile, bass, n_layers)
        nc.compile()
        _CACHE[key] = nc
    return _CACHE[key]


def kernel(**inputs):
    inputs = {k: np.asarray(v) for k, v in inputs.items()}
    n_layers = int(os.environ.get("GNN_LAYERS", NLAYERS))
    in_maps, mask = _make_in_maps(inputs, n_layers)
    nc = _get_nc(n_layers)

    if os.environ.get("GNN_SIM"):
        from concourse.bass_interp import CoreSim

        outs = []
        for b in range(int(os.environ.get("GNN_SIM_CORES", 1))):
            sim = CoreSim(nc, trace=False)
            for k, v in in_maps[b].items():
                sim.tensor(k)[:] = v
            sim.simulate()
            outs.append(np.array(sim.tensor("out")).reshape(NS, 1))
        while len(outs) < B:
            outs.append(np.zeros((NS, 1), _F32))
        out = np.stack(outs)
    else:
        from concourse.bass_utils import run_bass_kernel_spmd

        if os.environ.get("GNN_TRACE"):
            _install_ntff_hook()
            tmpdir = os.environ.get("GNN_TRACE_DIR") or None
            try:
                res = run_bass_kernel_spmd(
                    nc, in_maps, core_ids=list(range(B)), trace=True, tmpdir=tmpdir
                )
                kernel.last_exec_time_ns = res.exec_time_ns
            except Exception as e:
                print(f"[gnn] traced run failed ({e!r}); retrying untraced")
                res = run_bass_kernel_spmd(nc, in_maps, core_ids=list(range(B)))
        else:
            res = run_bass_kernel_spmd(nc, in_maps, core_ids=list(range(B)))
        kernel.last_results = res
        out = np.stack([r["out"].reshape(NS, 1) for r in res.results])

    return (out * inputs["node_mask"][:, :, None]).astype(_F32)
